# revision 3
# baseline (speedup 1.0000x reference)
"""PrefSimMat (EucDis mode) Trainium2 kernel.

sim[i,j] = 1 - dist[i,j] / ||dist[i,:]||_2,  dist = pairwise Euclidean
distance of the rows of p_u [8192, 256] fp32.

Strategy (8 NeuronCores, data-parallel over query rows):
  - Each core computes a [1024, 8192] tile of the output.
  - Gram-matrix identity: sq[i,j] = ni + nj - 2*g[i,j].  Features are
    quantized once to fp8e4 (e4m3); the Gram matrix is computed on TensorE
    in DoubleRow perf mode (one matmul contracts all 256 feature rows as
    128 partitions x 2 pairs at 0.5 cycles/output-col -- 3x fewer PE
    cycles than the bf16 3-chunk version).
  - The per-column nj term rides in a second K=2 DoubleRow matmul:
    nj - 256 = 16*hi + mid + lo/16 with hi/mid/lo fp8e4 rows (abs err
    ~4e-3).  The per-row terms ni + 256 + eps ride in the ScalarE
    activation bias, so no contraction rows are spent on them.
  - Row norms are computed analytically on the host (O(N*D)) from the
    quantized features, so device and host are numerically consistent:
    rowsum_i = N*(ni+eps) + sum_j nj_eff - 2 * a_i . (sum_j a_j).
  - ScalarE: t = Sqrt(psum * r2_i + r2_i*(ni+256+eps)) (per-partition
    scale/bias APs) = dist_ij/rownorm_i, written as fp16.
  - VectorE: out = t * (-1) + 1 (fp16 -> fp16, packed-2-byte fast mode).
  - Staged [128, 8192] fp16 rows DMA'd out as single 2 MiB transfers; host
    casts to fp32 (sim ~= 1, fp16 rounding ~2.5e-4 absmax, rel ~1.4e-4).
  - Input rhs is loaded in 2048-column groups so TensorE starts after the
    first group instead of after the full load.

Raw Bass (no TileContext): the walrus build in this container allows at most
one semaphore wait attached per compute instruction, so all cross-engine
dependencies are standalone wait_ge instructions with hand-rolled semaphores.
CoreSim race rule: every semaphore update crossing a waited threshold must be
ordered by its own issuing engine -> one semaphore per input DMA, and the
output-DMA stream uses parity-split semaphores with issuing-engine self-waits.
"""

import numpy as np
import ml_dtypes

F8 = ml_dtypes.float8_e4m3   # == mybir.dt.float8e4

N = 8192        # rows of p_u == output dim
D = 256         # feature dim
P = 128         # partitions
NCORES = 8
M_PER_CORE = N // NCORES       # 1024 output rows per core
MC = M_PER_CORE // P           # 8 m-chunks of 128 rows
NT = 512        # matmul free-dim tile (one PSUM bank fp32)
GW = 2048       # ACT/DVE group width = 4 PSUM banks
NG = N // GW    # 4 groups per m-chunk
EPS = 2.0 ** -3 # keeps sqrt argument positive on the diagonal under
                # PSUM/fp8-split rounding (device excursions ~0.01 observed)
CNJ = 256.0     # nj centering constant (absorbed into the ACT bias)

OUT_DT = np.float16

_CACHE = {}


def _build_nc():
    import concourse.bass as bass
    import concourse.mybir as mybir

    f32 = mybir.dt.float32
    f16 = mybir.dt.float16
    f8 = mybir.dt.float8e4
    AF = mybir.ActivationFunctionType
    ALU = mybir.AluOpType
    PM = mybir.MatmulPerfMode.DoubleRow

    nc = bass.Bass()
    lhsT_d = nc.dram_tensor("lhsT", [P, 2, M_PER_CORE], f8, kind="ExternalInput")
    rhs_d = nc.dram_tensor("rhs", [P, 2, N], f8, kind="ExternalInput")
    extw_d = nc.dram_tensor("extw", [2, 2, P], f8, kind="ExternalInput")
    extr_d = nc.dram_tensor("extr", [2, 2, N], f8, kind="ExternalInput")
    sc_d = nc.dram_tensor("sc", [P, 2 * MC], f32, kind="ExternalInput")
    out_d = nc.dram_tensor("out", [M_PER_CORE, N], f16, kind="ExternalOutput")

    NGI = MC * NG  # 32 pipeline groups

    from contextlib import ExitStack

    with ExitStack() as ctx:
        rhs_s = ctx.enter_context(nc.sbuf_tensor("rhs_s", [P, 2, N], f8))
        lhsT_s = ctx.enter_context(nc.sbuf_tensor("lhsT_s", [P, 2, M_PER_CORE], f8))
        extw_s = ctx.enter_context(nc.sbuf_tensor("extw_s", [2, 2, P], f8))
        extr_s = ctx.enter_context(nc.sbuf_tensor("extr_s", [2, 2, N], f8))
        sc_s = ctx.enter_context(nc.sbuf_tensor("sc_s", [P, 2 * MC], f32))
        tbuf = ctx.enter_context(nc.sbuf_tensor("tbuf", [P, 4 * GW], f16))
        stage = ctx.enter_context(nc.sbuf_tensor("stage", [P, 2 * N], f16))
        ps = ctx.enter_context(nc.psum_tensor("ps", [P, 2 * GW], f32))
        rhs_g_sems = [
            ctx.enter_context(nc.semaphore(f"in_rhs{g}")) for g in range(NG)
        ]
        in_l = ctx.enter_context(nc.semaphore("in_l"))
        in_ext = ctx.enter_context(nc.semaphore("in_ext"))
        in_sc = ctx.enter_context(nc.semaphore("in_sc"))
        sem_mm = ctx.enter_context(nc.semaphore("sem_mm"))
        sem_act = ctx.enter_context(nc.semaphore("sem_act"))
        sem_ts = ctx.enter_context(nc.semaphore("sem_ts"))
        dma_out0 = ctx.enter_context(nc.semaphore("dma_out0"))
        dma_out1 = ctx.enter_context(nc.semaphore("dma_out1"))
        block = ctx.enter_context(nc.Block())
        out_sems = [dma_out0, dma_out1]

        @block.sync
        def _(sync):
            sync.dma_start(sc_s[:, :], sc_d[:, :]).then_inc(in_sc, 16)
            sync.dma_start(lhsT_s[:, :, :], lhsT_d[:, :, :]).then_inc(in_l, 16)
            sync.dma_start(extw_s[:, :, :], extw_d[:, :, :]).then_inc(in_ext, 16)
            sync.dma_start(extr_s[:, :, :], extr_d[:, :, :]).then_inc(in_ext, 16)
            for g in range(NG):
                c0, c1 = g * GW, (g + 1) * GW
                sync.dma_start(
                    rhs_s[:, :, c0:c1], rhs_d[:, :, c0:c1]
                ).then_inc(rhs_g_sems[g], 16)
            for m in range(MC):
                sync.wait_ge(sem_ts, (m + 1) * NG)
                if m >= 2:
                    # serialize increments of the parity sem (2 DMAs in flight)
                    sync.wait_ge(out_sems[m % 2], 16 * (m // 2))
                sync.dma_start(
                    out_d[m * P : (m + 1) * P, :],
                    stage[:, (m % 2) * N : (m % 2 + 1) * N],
                ).then_inc(out_sems[m % 2], 16)

        @block.tensor
        def _(tensor):
            tensor.wait_ge(in_l, 16)
            tensor.wait_ge(in_ext, 32)
            for m in range(MC):
                lsl = lhsT_s[:, :, m * P : (m + 1) * P]
                for g in range(NG):
                    gi = m * NG + g
                    if m == 0:
                        tensor.wait_ge(rhs_g_sems[g], 16)
                    if gi >= 2:
                        tensor.wait_ge(sem_act, gi - 1)
                    inst = None
                    for j in range(GW // NT):
                        n0 = g * GW + j * NT
                        p0 = (gi % 2) * GW + j * NT
                        tensor.matmul(
                            ps[:, p0 : p0 + NT],
                            lsl,
                            rhs_s[:, :, n0 : n0 + NT],
                            start=True,
                            stop=False,
                            perf_mode=PM,
                        )
                        inst = tensor.matmul(
                            ps[:, p0 : p0 + NT],
                            extw_s[:, :, :],
                            extr_s[:, :, n0 : n0 + NT],
                            start=False,
                            stop=True,
                            perf_mode=PM,
                        )
                    inst.then_inc(sem_mm, 1)

        @block.scalar
        def _(scalar):
            scalar.wait_ge(in_sc, 16)
            for gi in range(NGI):
                m = gi // NG
                scalar.wait_ge(sem_mm, gi + 1)
                if gi >= 4:
                    scalar.wait_ge(sem_ts, gi - 3)
                scalar.activation(
                    tbuf[:, (gi % 4) * GW : (gi % 4 + 1) * GW],
                    ps[:, (gi % 2) * GW : (gi % 2 + 1) * GW],
                    AF.Sqrt,
                    scale=sc_s[:, m : m + 1],
                    bias=sc_s[:, MC + m : MC + m + 1],
                ).then_inc(sem_act, 1)

        @block.vector
        def _(vector):
            for gi in range(NGI):
                m, g = divmod(gi, NG)
                vector.wait_ge(sem_act, gi + 1)
                if g == 0 and m >= 2:
                    vector.wait_ge(out_sems[m % 2], 16 * (m // 2))
                vector.tensor_scalar(
                    stage[:, (m % 2) * N + g * GW : (m % 2) * N + (g + 1) * GW],
                    tbuf[:, (gi % 4) * GW : (gi % 4 + 1) * GW],
                    -1.0,
                    1.0,
                    op0=ALU.mult,
                    op1=ALU.add,
                ).then_inc(sem_ts, 1)

    return nc


def _prep_inputs(p_u):
    """Host-side O(N*D) prep: fp8 cast/transpose, norms, row sums."""
    a8 = p_u.astype(F8)                       # quantize features once
    af = a8.astype(np.float32)
    a64 = af.astype(np.float64)
    ni64 = np.einsum("ij,ij->i", a64, a64)    # [N] norms of quantized rows

    # nj extension rows: nj - CNJ = 16*hi + mid + lo/16 (fp8e4 splits)
    njp = ni64 - CNJ
    hi8 = (njp / 16.0).astype(np.float32).astype(F8)
    hi = hi8.astype(np.float64)
    r = njp - 16.0 * hi
    mid8 = r.astype(np.float32).astype(F8)
    mid = mid8.astype(np.float64)
    lo8 = (16.0 * (r - mid)).astype(np.float32).astype(F8)
    lo = lo8.astype(np.float64)
    nj_eff = CNJ + 16.0 * hi + mid + lo / 16.0

    t64 = a64.sum(axis=0)                     # [D]
    rowsum = N * ni64 + nj_eff.sum() - 2.0 * (a64 @ t64) + N * EPS
    r2 = 1.0 / rowsum                         # [N] f64
    bias64 = r2 * (ni64 + CNJ + EPS)

    rhs = np.ascontiguousarray(
        a8.T.reshape(2, P, N).transpose(1, 0, 2)
    )                                         # [128, 2, 8192] fp8
    extr = np.zeros((2, 2, N), dtype=F8)
    extr[0, 0] = hi8
    extr[0, 1] = mid8
    extr[1, 0] = lo8
    extw = np.zeros((2, 2, P), dtype=F8)
    extw[0, 0, :] = F8(16.0)
    extw[0, 1, :] = F8(1.0)
    extw[1, 0, :] = F8(1.0 / 16.0)

    m2 = (-2.0 * af).astype(F8)               # exact fp8 doubling
    r2f = r2.astype(np.float32)
    biasf = bias64.astype(np.float32)

    in_maps = []
    for c in range(NCORES):
        sl = slice(c * M_PER_CORE, (c + 1) * M_PER_CORE)
        lhsT = np.ascontiguousarray(
            m2[sl].T.reshape(2, P, M_PER_CORE).transpose(1, 0, 2)
        )                                     # [128, 2, 1024] fp8
        sc = np.concatenate(
            [
                np.ascontiguousarray(r2f[sl].reshape(MC, P).T),
                np.ascontiguousarray(biasf[sl].reshape(MC, P).T),
            ],
            axis=1,
        ).astype(np.float32)                  # [128, 16]
        in_maps.append(
            {"lhsT": lhsT, "rhs": rhs, "extw": extw, "extr": extr, "sc": sc}
        )
    return in_maps


def kernel(p_u):
    from concourse.bass_utils import run_bass_kernel_spmd

    p_u = np.asarray(p_u, dtype=np.float32)
    assert p_u.shape == (N, D)

    if "nc" not in _CACHE:
        _CACHE["nc"] = _build_nc()
    nc = _CACHE["nc"]

    in_maps = _prep_inputs(p_u)
    trace = bool(_CACHE.get("trace"))
    res = run_bass_kernel_spmd(nc, in_maps, core_ids=list(range(NCORES)), trace=trace)
    _CACHE["last_result"] = res
    out = np.concatenate(
        [res.results[c]["out"].astype(np.float32) for c in range(NCORES)], axis=0
    )
    return out
